# revision 19
# baseline (speedup 1.0000x reference)
"""Trainium2 Bass kernel for the AggregateLayer pooling problem.

reference semantics (per batch b):
    dot_w[j] = <pref[b,j,:], c[b,0,:]>                      (j = 0..63)
    t_w[j]   = 1 / |t_pref[b,0,j] - t_c[b,0]|
    w        = softmax(dot_w + t_w)                          (over j)
    u[b,0,:] = sum_j w[j] * pref[b,j,:]

Strategy: pure data parallel over 8 NeuronCores (1024 batches each).
Per core, batches are processed in groups of GROUP=128 (NTILES=64 tiles
of 2 batches; a tile is the 128 flattened (batch, j) rows x 128 D cols).

The kernel is HBM-bandwidth-bound: the pref stream (33.5 MB fp32/core)
runs at the ~360 GB/s per-core HBM cap (~95 us of pure streaming).
Everything else is engineered to hide under that stream:
  - pref chunk loads (cast fp32->fp16 in SWDGE) are the only traffic on
    the gpsimd ring; c/t_pref/t_c loads and u stores ride sync HWDGE.
  - group 0's c slice loads first (contiguous 64 KB) so its dot matmuls
    aren't gated; later groups' c transposes are emitted lazily; the
    identity build (gpsimd-only) slots behind group 0's first chunk.
  - engine streams are in-order, so emission is software-pipelined at
    COARSE block granularity (fine interleaving measurably costs ~10ns
    of semaphore latency per matmul): dot-matmul chunks trail the
    transpose chunks by LAG=3 (both paced by the PSUM->SBUF pts copies,
    split 5:3 ACT/DVE), and each group's weighted sum is deferred one
    full group so PE chews on group g+1's transposes while group g's
    softmax runs on DVE/ACT.
  - u is stored transposed [D, BPC] straight from the weighted-sum PSUM
    layout (saves two PE transposes + copies per group); the host
    transposes it back. W_MAT lives in a persistent pre-zeroed 3-ring.
  - t_w is computed on ACT (Abs with bias=-t_c, ntca precomputed) and
    wn16 on ACT (Copy with per-partition scale=rz), freeing ~2us/group
    of DVE time on the softmax->W_MAT critical chain.

Approaches measured SLOWER this hardware/stack (do not retry blindly):
  - XBAR dma_start_transpose (sync HWDGE) to replace PE transposes:
    the runtime serializes HWDGE XPOSEs against the SWDGE pref stream
    with ~2-12us ring handoffs (and concurrent XPOSEs on the two HWDGE
    queues corrupt each other's output). Any XPOSE/SWDGE mix ran
    224-324us. XPOSE alone streams at ~190 GB/s per queue.
  - batch-on-partitions dataflow (dots/softmax/wsum as within-partition
    DVE ops with stride-0 broadcast APs): DVE runs 1x (not 2x) on
    inner-stride-0 broadcast operands and on tensor_reduce, so a
    128-batch block costs ~26us of DVE vs ~11us of PE on the row path;
    a 6/2 row/batch split measured 155us.
  - tiny N=2 matmuls cost ~36ns back-to-back (LDWEIGHTS pipelines);
    PE transposes ~95ns; the SWDGE pref stream delivers ~712 GB/s.

Tuning cliffs measured on hardware (do not "fix" these without
re-measuring): the last group's HT=8 DMA chunking is load-bearing
(HT=16 there reproducibly ~2x-es runtime via p16 buffer-ring
interaction); per-half softmax chains, fully-upfront DMA emission, and
fp32-bitcast PSUM copies (PSUM fp16 is not packed-viewable) all
regressed. Run-to-run noise is +-3 us with occasional +15-40 us
device-state excursions -- attribute changes only across repeated runs.
"""

import numpy as np
from contextlib import ExitStack

import concourse.bass as bass
import concourse.tile as tile
from concourse import mybir
from concourse.masks import make_identity
from concourse.bass_utils import run_bass_kernel_spmd
import concourse.bass2jax as _b2j


def _split_multiwait(bir: dict) -> int:
    """Walrus in this container rejects >1 sync-wait per instruction.

    Hoist excess waits onto NoOps inserted just before the instruction on
    the same engine (program order within the engine stream preserves the
    wait semantics exactly).
    """
    n = 0
    for fn in bir["functions"]:
        for blk in fn["blocks"]:
            out = []
            for inst in blk["instructions"]:
                si = inst.get("sync_info")
                waits = si.get("on_wait") if si else None
                if waits and len(waits) > 1:
                    for w in waits[:-1]:
                        out.append(
                            {
                                "opcode": "NoOp",
                                "engine": inst["engine"],
                                "name": f"{inst['name']}-xw{n}",
                                "ins": [],
                                "outs": [],
                                "sync_info": {"on_update": [], "on_wait": [w]},
                            }
                        )
                        n += 1
                    si["on_wait"] = [waits[-1]]
                out.append(inst)
            blk["instructions"] = out
    return n


_orig_compile_bir_kernel = _b2j.compile_bir_kernel


def _legalizing_compile_bir_kernel(ant_bir_str, *args, **kwargs):
    import orjson

    bir = orjson.loads(ant_bir_str)
    _split_multiwait(bir)
    return _orig_compile_bir_kernel(orjson.dumps(bir), *args, **kwargs)


_b2j.compile_bir_kernel = _legalizing_compile_bir_kernel

F32 = mybir.dt.float32
F16 = mybir.dt.float16
Alu = mybir.AluOpType
Act = mybir.ActivationFunctionType
Axis = mybir.AxisListType

B, N, D = 8192, 64, 128
NCORES = 8
BPC = B // NCORES          # 1024 batches per core
GROUP = 128                # batches per group
NGROUPS = BPC // GROUP     # 8
NTILES = GROUP // 2        # 64 two-batch tiles per group
NPAIR = GROUP // 2         # 64 batch-pairs per group (softmax partitions)
NH = GROUP // 128          # 1 c-half per group
CH = 8                     # tiles per transpose/copy chunk
NCH = NTILES // CH         # 8 chunks per group
LAG = 3                    # dot-chunk k trails transpose-chunk k+LAG


class _St:
    """Per-group pipeline state carried between build phases."""

    def __init__(self, g):
        self.g = g
        self.p16 = None
        self.tw = None
        self.w = None
        self.nmx = None
        self.wn16 = None
        self.wmat16 = None


class _Ctx:
    def __init__(self, tc, pools, consts, aps):
        self.tc = tc
        self.nc = tc.nc
        (self.p_p16, self.p_pt, self.p_small, self.ps_pt, self.ps_mm,
         self.ps_small) = pools
        self.ident16, self.ident32 = consts
        (self.pref_rows, self.u_all, self.ct16a, self.tpa, self.tca,
         self.ntca, self.cg16, self.wmat_ring) = aps


def _phase_a(cx, g, p16, nxt_p16):
    """tw + interleaved transpose/dot passes + dots extraction + add/max."""
    nc = cx.nc
    st = _St(g)
    st.p16 = p16

    # pref chunk DMAs for this group. Chunk 0 (16 tiles) was prefetched
    # by the previous iteration (or the prelude for g=0), killing the
    # ~1.4us chunk-0 load latency at each iteration boundary; last group
    # uses smaller chunks so the drain after the final chunk is short
    r0 = g * GROUP * N
    HT = 16 if g < NGROUPS - 1 else 8
    for h0 in range(16, NTILES, HT):
        rh = r0 + h0 * 128
        nc.gpsimd.dma_start(
            out=p16[:, h0 : h0 + HT, :],
            in_=cx.pref_rows[rh : rh + HT * 128, :].rearrange(
                "(t p) d -> p t d", p=128
            ),
        )
    if nxt_p16 is not None:
        rn = (g + 1) * GROUP * N
        nc.gpsimd.dma_start(
            out=nxt_p16[:, 0:16, :],
            in_=cx.pref_rows[rn : rn + 16 * 128, :].rearrange(
                "(t p) d -> p t d", p=128
            ),
        )

    st.tw = cx.p_small.tile([NPAIR, 2, N], F32, tag="tw", name=f"tw{g}")
    for s in range(2):
        nc.scalar.activation(
            out=st.tw[:, s, :],
            in_=cx.tpa[:, g, s, :],
            func=Act.Abs,
            bias=cx.ntca[:, g, s : s + 1],
            scale=1.0,
        )
    nc.vector.reciprocal(out=st.tw[:], in_=st.tw[:])

    pts = cx.p_pt.tile([128, NTILES, 128], F16, tag="pts", name=f"pts{g}")
    ps_dots = cx.ps_mm.tile(
        [128, NTILES, 2], F32, tag="mm_ps", name=f"dots{g}"
    )

    def t_chunk(k):
        t0 = k * CH
        pt_ps = cx.ps_pt.tile(
            [128, CH, 128], F16, tag="pt_ps", name=f"ptps{g}_{k}"
        )
        for i in range(CH):
            nc.tensor.transpose(
                out=pt_ps[:, i, :],
                in_=p16[:, t0 + i, :],
                identity=cx.ident16[:],
            )
        if k < 4:
            nc.scalar.copy(out=pts[:, t0 : t0 + CH, :], in_=pt_ps[:])
        else:
            nc.vector.tensor_copy(out=pts[:, t0 : t0 + CH, :], in_=pt_ps[:])

    def d_chunk(k):
        t0 = k * CH
        for i in range(CH):
            t = t0 + i
            th, tr = divmod(t, 128 // 2)
            nc.tensor.matmul(
                out=ps_dots[:, t, :],
                lhsT=pts[:, t, :],
                rhs=cx.ct16a[:, NH * g + th, 2 * tr : 2 * tr + 2],
                start=(i == 0),
                stop=(i == CH - 1),
            )

    for k in range(NCH):
        t_chunk(k)
        if k >= LAG:
            d_chunk(k - LAG)
    for k in range(NCH - LAG, NCH):
        d_chunk(k)

    # valid dots sit at [row, parity=row//64]: extract the two halves
    dotw = cx.p_small.tile([128, NTILES], F32, tag="dotw", name=f"dotw{g}")
    nc.scalar.copy(out=dotw[0:64, :], in_=ps_dots[0:64, :, 0])
    nc.scalar.copy(out=dotw[64:128, :], in_=ps_dots[64:128, :, 1])

    # transpose [128(row), nt] -> [nt, 128(row)] => pair-major dots
    dr_ps = cx.ps_small.tile([NPAIR, 128], F32, tag="sm_ps", name=f"dr{g}")
    nc.tensor.transpose(out=dr_ps[:], in_=dotw[:], identity=cx.ident32[:])

    st.w = cx.p_small.tile([NPAIR, 2, N], F32, tag="w", name=f"w{g}")
    nc.vector.tensor_add(
        out=st.w[:],
        in0=dr_ps[:].rearrange("t (two n) -> t two n", two=2),
        in1=st.tw[:],
    )
    st.nmx = cx.p_small.tile([NPAIR, 2], F32, tag="nmx", name=f"nmx{g}")
    nc.vector.tensor_reduce(
        out=st.nmx[:], in_=st.w[:], axis=Axis.X, op=Alu.max, negate=True
    )
    _phase_b1(cx, st)
    return st


def _phase_b1(cx, st):
    """Softmax tail: exp + sum + reciprocal + normalize (no PE)."""
    nc = cx.nc
    g = st.g
    e = cx.p_small.tile([NPAIR, 2, N], F32, tag="e", name=f"e{g}")
    for s in range(2):
        nc.scalar.activation(
            out=e[:, s, :],
            in_=st.w[:, s, :],
            func=Act.Exp,
            bias=st.nmx[:, s : s + 1],
            scale=1.0,
        )
    z = cx.p_small.tile([NPAIR, 2], F32, tag="z", name=f"z{g}")
    nc.vector.reduce_sum(out=z[:], in_=e[:], axis=Axis.X)
    rz = cx.p_small.tile([NPAIR, 2], F32, tag="rz", name=f"rz{g}")
    nc.vector.reciprocal(out=rz[:], in_=z[:])
    st.wn16 = cx.p_small.tile([NPAIR, 2, N], F16, tag="wn16", name=f"wn{g}")
    for s in range(2):
        nc.scalar.activation(
            out=st.wn16[:, s, :],
            in_=e[:, s, :],
            func=Act.Copy,
            scale=rz[:, s : s + 1],
        )


def _phase_b2(cx, st):
    """W_MAT build: PE transpose of wn16 + block scatter."""
    nc = cx.nc
    g = st.g
    wc_ps = cx.ps_small.tile([128, NTILES], F16, tag="sm_ps", name=f"wc{g}")
    nc.tensor.transpose(
        out=wc_ps[:],
        in_=st.wn16[:].rearrange("t two n -> t (two n)"),
        identity=cx.ident16[0:NPAIR, 0:NPAIR],
    )
    wcol = cx.p_small.tile([128, NTILES], F16, tag="wcol", name=f"wcol{g}")
    nc.vector.tensor_copy(out=wcol[:], in_=wc_ps[:])
    # persistent pre-zeroed ring: only the data halves are ever written,
    # the zero halves survive across generations
    st.wmat16 = cx.wmat_ring[g % len(cx.wmat_ring)]
    nc.vector.tensor_copy(out=st.wmat16[0:64, :, 0], in_=wcol[0:64, :])
    nc.vector.tensor_copy(out=st.wmat16[64:128, :, 1], in_=wcol[64:128, :])


def _phase_c(cx, st):
    """Weighted-sum matmuls + u extraction (ACT) + store (sync)."""
    nc = cx.nc
    g = st.g
    b0 = g * GROUP
    HB = NTILES // 2
    for h in range(2):
        ps_ut = cx.ps_mm.tile(
            [128, HB, 2], F32, tag="mm_ps", name=f"ut{g}_{h}"
        )
        for k in range(HB):
            t = h * HB + k
            nc.tensor.matmul(
                out=ps_ut[:, k, :],
                lhsT=st.p16[:, t, :],
                rhs=st.wmat16[:, t, :],
                start=(k == 0),
                stop=(k == HB - 1),
            )
        uts = cx.p_small.tile(
            [128, GROUP // 2], F32, tag="uts", name=f"uts{g}_{h}"
        )
        nc.vector.tensor_copy(
            out=uts[:], in_=ps_ut[:].rearrange("d t two -> d (t two)")
        )
        bh = b0 + h * (GROUP // 2)
        nc.sync.dma_start(
            out=cx.u_all[:, bh : bh + GROUP // 2], in_=uts[:]
        )


def _emit_ct(cx, g):
    """PE transposes of group g's c halves into ct16a (+ scalar copies)."""
    nc = cx.nc
    for h in range(NH):
        gh = NH * g + h
        ct_ps = cx.ps_small.tile([128, 128], F16, tag="sm_ps", name=f"ct{gh}")
        nc.tensor.transpose(
            out=ct_ps[:],
            in_=cx.cg16[:, gh, :],
            identity=cx.ident16[:],
        )
        nc.vector.tensor_copy(out=cx.ct16a[:, gh, :], in_=ct_ps[:])


def _build_nc():
    nc = bass.Bass()
    pref = nc.declare_dram_parameter("pref", [BPC, N, D], F32, isOutput=False)
    c = nc.declare_dram_parameter("c", [BPC, 1, D], F32, isOutput=False)
    t_pref = nc.declare_dram_parameter("t_pref", [BPC, 1, N], F32, isOutput=False)
    t_c = nc.declare_dram_parameter("t_c", [BPC, 1], F32, isOutput=False)
    # u stored transposed [D, BPC] (direct from the weighted-sum PSUM
    # layout -- skips two PE transposes + copies per group); the host
    # transposes it back.
    u = nc.declare_dram_parameter("u", [D, BPC], F32, isOutput=True)

    pref_rows = pref[:].rearrange("b n d -> (b n) d")
    c_all = c[:].rearrange("b one d -> (b one) d")
    tp_all = t_pref[:].rearrange("b one n -> (b one) n")
    tc_all = t_c[:]
    u_all = u[:]

    with ExitStack() as ctx:
        tc = ctx.enter_context(tile.TileContext(nc))
        p_const = ctx.enter_context(tc.tile_pool(name="const", bufs=1))
        p_pre = ctx.enter_context(tc.tile_pool(name="pre", bufs=1))
        p_p16 = ctx.enter_context(tc.tile_pool(name="p16", bufs=4))
        p_pt = ctx.enter_context(tc.tile_pool(name="pt", bufs=3))
        p_small = ctx.enter_context(tc.tile_pool(name="small", bufs=3))
        ps_pt = ctx.enter_context(tc.tile_pool(name="ps_pt", bufs=3, space="PSUM"))
        ps_mm = ctx.enter_context(tc.tile_pool(name="ps_mm", bufs=3, space="PSUM"))
        ps_small = ctx.enter_context(
            tc.tile_pool(name="ps_small", bufs=2, space="PSUM")
        )

        nb = NGROUPS * GROUP

        # sync stream head: group 0's c first (contiguous 128 KB -- it
        # gates group 0's dot matmuls), then t tensors, then the rest of c.
        NGH = NGROUPS * NH
        c32a = p_pre.tile([128, NGH, D], F32)
        nc.sync.dma_start(
            out=c32a[:, 0:NH, :],
            in_=c_all[0:GROUP, :].rearrange("(h b) d -> b h d", b=128),
        )
        tpa = p_pre.tile([NPAIR, NGROUPS, 2, N], F32)
        nc.sync.dma_start(
            out=tpa[:],
            in_=tp_all[0:nb, :].rearrange(
                "(g t two) n -> t g two n", t=NPAIR, two=2
            ),
        )
        tca = p_pre.tile([NPAIR, NGROUPS, 2], F32)
        nc.sync.dma_start(
            out=tca[:],
            in_=tc_all[0:nb, :].rearrange(
                "(g t two) one -> t g (two one)", t=NPAIR, two=2
            ),
        )
        nc.sync.dma_start(
            out=c32a[:, NH:, :],
            in_=c_all[GROUP:nb, :].rearrange("(g b) d -> b g d", b=128),
        )

        ntca = p_pre.tile([NPAIR, NGROUPS, 2], F32)
        nc.vector.tensor_scalar_mul(out=ntca[:], in0=tca[:], scalar1=-1.0)

        # gpsimd stream: group 0's first pref chunk, then the identity
        # build (gpsimd-only affine_select) -- identities are ready right
        # when chunk 0's data lands.
        p16s = []
        for _gi in range(NGROUPS):
            p16_t = p_p16.tile(
                [128, NTILES, D], F16, tag="p16", name=f"p16_{_gi}"
            )
            p16s.append(p16_t)
        nc.gpsimd.dma_start(
            out=p16s[0][:, 0:16, :],
            in_=pref_rows[0 : 16 * 128, :].rearrange("(t p) d -> p t d", p=128),
        )
        ident16 = p_const.tile([128, 128], F16)
        ident32 = p_const.tile([128, 128], F32)
        make_identity(nc, ident16[:])
        make_identity(nc, ident32[:])
        consts = (ident16, ident32)

        # c cast: group 0 first, rest later (gates nothing early)
        cg16 = p_pre.tile([128, NGH, D], F16)
        nc.vector.tensor_copy(out=cg16[:, 0:NH, :], in_=c32a[:, 0:NH, :])
        nc.vector.tensor_copy(out=cg16[:, NH:, :], in_=c32a[:, NH:, :])
        ct16a = p_pre.tile([128, NGH, 128], F16)  # [D, group-half, batch]

        wmat_ring = []
        for _wi in range(3):
            wm = p_pre.tile([128, NTILES, 2], F16, name=f"wmatr{_wi}")
            nc.vector.memset(wm[:], 0.0)
            wmat_ring.append(wm)

        aps = (pref_rows, u_all, ct16a, tpa, tca, ntca, cg16, wmat_ring)
        cx = _Ctx(tc, (p_p16, p_pt, p_small, ps_pt, ps_mm, ps_small),
                  consts, aps)

        _emit_ct(cx, 0)

        # software pipeline, coarse blocks (fine interleaving costs ~10ns
        # of extra semaphore latency per matmul -- measured):
        #   iter g: [softmax-tail g-1] [A: loads+transposes+dots g]
        #           [W_MAT g-1] [weighted-sum + store g-1] [cT g+1]
        # Each deferred block's inputs are ready ~7 us before PE reaches
        # it, so no engine queue ever waits mid-chain.
        pend = None
        for g in range(NGROUPS):
            st = _phase_a(cx, g, p16s[g],
                          p16s[g + 1] if g + 1 < NGROUPS else None)
            if pend is not None:
                _phase_c(cx, pend)
            _phase_b2(cx, st)
            if g + 1 < NGROUPS:
                _emit_ct(cx, g + 1)
            pend = st

        _phase_c(cx, pend)

    return nc


_NC_CACHE = None
LAST_RESULT = None


def kernel(pref, c, t_pref, t_c):
    global _NC_CACHE, LAST_RESULT
    if _NC_CACHE is None:
        _NC_CACHE = _build_nc()
    nc = _NC_CACHE

    pref = np.ascontiguousarray(pref, dtype=np.float32)
    c = np.ascontiguousarray(c, dtype=np.float32)
    t_pref = np.ascontiguousarray(t_pref, dtype=np.float32)
    t_c = np.ascontiguousarray(t_c, dtype=np.float32)

    in_maps = []
    for i in range(NCORES):
        s = slice(i * BPC, (i + 1) * BPC)
        in_maps.append(
            {"pref": pref[s], "c": c[s], "t_pref": t_pref[s], "t_c": t_c[s]}
        )

    res = run_bass_kernel_spmd(nc, in_maps, list(range(NCORES)))
    LAST_RESULT = res
    return np.ascontiguousarray(
        np.concatenate([r["u"].T for r in res.results], axis=0)
    ).reshape(B, 1, D)



# revision 20
# speedup vs baseline: 1.0002x; 1.0002x over previous
"""Trainium2 Bass kernel for the AggregateLayer pooling problem.

reference semantics (per batch b):
    dot_w[j] = <pref[b,j,:], c[b,0,:]>                      (j = 0..63)
    t_w[j]   = 1 / |t_pref[b,0,j] - t_c[b,0]|
    w        = softmax(dot_w + t_w)                          (over j)
    u[b,0,:] = sum_j w[j] * pref[b,j,:]

Strategy: pure data parallel over 8 NeuronCores (1024 batches each).
Per core, batches are processed in groups of GROUP=128 (NTILES=64 tiles
of 2 batches; a tile is the 128 flattened (batch, j) rows x 128 D cols).

The kernel is HBM-bandwidth-bound: the pref stream (33.5 MB fp32/core)
runs at the ~360 GB/s per-core HBM cap (~95 us of pure streaming).
Everything else is engineered to hide under that stream:
  - pref chunk loads (cast fp32->fp16 in SWDGE) are the only traffic on
    the gpsimd ring; c/t_pref/t_c loads and u stores ride sync HWDGE.
  - group 0's c slice loads first (contiguous 64 KB) so its dot matmuls
    aren't gated; later groups' c transposes are emitted lazily; the
    identity build (gpsimd-only) slots behind group 0's first chunk.
  - engine streams are in-order, so emission is software-pipelined at
    COARSE block granularity (fine interleaving measurably costs ~10ns
    of semaphore latency per matmul): dot-matmul chunks trail the
    transpose chunks by LAG=3 (both paced by the PSUM->SBUF pts copies,
    split 5:3 ACT/DVE), and each group's weighted sum is deferred one
    full group so PE chews on group g+1's transposes while group g's
    softmax runs on DVE/ACT.
  - u is stored transposed [D, BPC] straight from the weighted-sum PSUM
    layout (saves two PE transposes + copies per group); the host
    transposes it back. W_MAT lives in a persistent pre-zeroed 3-ring.
  - t_w is computed on ACT (Abs with bias=-t_c, ntca precomputed) and
    wn16 on ACT (Copy with per-partition scale=rz), freeing ~2us/group
    of DVE time on the softmax->W_MAT critical chain.

Approaches measured SLOWER this hardware/stack (do not retry blindly):
  - XBAR dma_start_transpose (sync HWDGE) to replace PE transposes:
    the runtime serializes HWDGE XPOSEs against the SWDGE pref stream
    with ~2-12us ring handoffs (and concurrent XPOSEs on the two HWDGE
    queues corrupt each other's output). Any XPOSE/SWDGE mix ran
    224-324us. XPOSE alone streams at ~190 GB/s per queue.
  - batch-on-partitions dataflow (dots/softmax/wsum as within-partition
    DVE ops with stride-0 broadcast APs): DVE runs 1x (not 2x) on
    inner-stride-0 broadcast operands and on tensor_reduce, so a
    128-batch block costs ~26us of DVE vs ~11us of PE on the row path;
    a 6/2 row/batch split measured 155us.
  - tiny N=2 matmuls cost ~36ns back-to-back (LDWEIGHTS pipelines);
    PE transposes ~95ns; the SWDGE pref stream delivers ~712 GB/s.

Tuning cliffs measured on hardware (do not "fix" these without
re-measuring): the last group's HT=8 DMA chunking is load-bearing
(HT=16 there reproducibly ~2x-es runtime via p16 buffer-ring
interaction); per-half softmax chains, fully-upfront DMA emission, and
fp32-bitcast PSUM copies (PSUM fp16 is not packed-viewable) all
regressed. Run-to-run noise is +-3 us with occasional +15-40 us
device-state excursions -- attribute changes only across repeated runs.
"""

import numpy as np
from contextlib import ExitStack

import concourse.bass as bass
import concourse.tile as tile
from concourse import mybir
from concourse.masks import make_identity
from concourse.bass_utils import run_bass_kernel_spmd
import concourse.bass2jax as _b2j


def _split_multiwait(bir: dict) -> int:
    """Walrus in this container rejects >1 sync-wait per instruction.

    Hoist excess waits onto NoOps inserted just before the instruction on
    the same engine (program order within the engine stream preserves the
    wait semantics exactly).
    """
    n = 0
    for fn in bir["functions"]:
        for blk in fn["blocks"]:
            out = []
            for inst in blk["instructions"]:
                si = inst.get("sync_info")
                waits = si.get("on_wait") if si else None
                if waits and len(waits) > 1:
                    for w in waits[:-1]:
                        out.append(
                            {
                                "opcode": "NoOp",
                                "engine": inst["engine"],
                                "name": f"{inst['name']}-xw{n}",
                                "ins": [],
                                "outs": [],
                                "sync_info": {"on_update": [], "on_wait": [w]},
                            }
                        )
                        n += 1
                    si["on_wait"] = [waits[-1]]
                out.append(inst)
            blk["instructions"] = out
    return n


_orig_compile_bir_kernel = _b2j.compile_bir_kernel


def _legalizing_compile_bir_kernel(ant_bir_str, *args, **kwargs):
    import orjson

    bir = orjson.loads(ant_bir_str)
    _split_multiwait(bir)
    return _orig_compile_bir_kernel(orjson.dumps(bir), *args, **kwargs)


_b2j.compile_bir_kernel = _legalizing_compile_bir_kernel

F32 = mybir.dt.float32
F16 = mybir.dt.float16
Alu = mybir.AluOpType
Act = mybir.ActivationFunctionType
Axis = mybir.AxisListType

B, N, D = 8192, 64, 128
NCORES = 8
BPC = B // NCORES          # 1024 batches per core
GROUP = 128                # batches per group
NGROUPS = BPC // GROUP     # 8
NTILES = GROUP // 2        # 64 two-batch tiles per group
NPAIR = GROUP // 2         # 64 batch-pairs per group (softmax partitions)
NH = GROUP // 128          # 1 c-half per group
CH = 8                     # tiles per transpose/copy chunk
NCH = NTILES // CH         # 8 chunks per group
LAG = 3                    # dot-chunk k trails transpose-chunk k+LAG


class _St:
    """Per-group pipeline state carried between build phases."""

    def __init__(self, g):
        self.g = g
        self.p16 = None
        self.tw = None
        self.w = None
        self.nmx = None
        self.wn16 = None
        self.wmat16 = None


class _Ctx:
    def __init__(self, tc, pools, consts, aps):
        self.tc = tc
        self.nc = tc.nc
        (self.p_p16, self.p_pt, self.p_small, self.ps_pt, self.ps_mm,
         self.ps_small) = pools
        self.ident16, self.ident32 = consts
        (self.pref_rows, self.u_all, self.ct16a, self.tpa, self.tca,
         self.ntca, self.cg16, self.wmat_ring) = aps


def _phase_a(cx, g, p16):
    """tw + interleaved transpose/dot passes + dots extraction + add/max."""
    nc = cx.nc
    st = _St(g)
    st.p16 = p16

    # pref chunk DMAs for this group (group 0's first chunk was emitted
    # ahead of the identity build in _build_nc); last group uses smaller
    # chunks so the drain after the final chunk is short
    r0 = g * GROUP * N
    HT = 16 if g < NGROUPS - 1 else 8
    for h0 in range(16 if g == 0 else 0, NTILES, HT):
        rh = r0 + h0 * 128
        nc.gpsimd.dma_start(
            out=p16[:, h0 : h0 + HT, :],
            in_=cx.pref_rows[rh : rh + HT * 128, :].rearrange(
                "(t p) d -> p t d", p=128
            ),
        )

    st.tw = cx.p_small.tile([NPAIR, 2, N], F32, tag="tw", name=f"tw{g}")
    for s in range(2):
        nc.scalar.activation(
            out=st.tw[:, s, :],
            in_=cx.tpa[:, g, s, :],
            func=Act.Abs,
            bias=cx.ntca[:, g, s : s + 1],
            scale=1.0,
        )
    nc.vector.reciprocal(out=st.tw[:], in_=st.tw[:])

    pts = cx.p_pt.tile([128, NTILES, 128], F16, tag="pts", name=f"pts{g}")
    ps_dots = cx.ps_mm.tile(
        [128, NTILES, 2], F32, tag="mm_ps", name=f"dots{g}"
    )

    def t_chunk(k):
        t0 = k * CH
        pt_ps = cx.ps_pt.tile(
            [128, CH, 128], F16, tag="pt_ps", name=f"ptps{g}_{k}"
        )
        for i in range(CH):
            nc.tensor.transpose(
                out=pt_ps[:, i, :],
                in_=p16[:, t0 + i, :],
                identity=cx.ident16[:],
            )
        if k < 5:
            nc.scalar.copy(out=pts[:, t0 : t0 + CH, :], in_=pt_ps[:])
        else:
            nc.vector.tensor_copy(out=pts[:, t0 : t0 + CH, :], in_=pt_ps[:])

    def d_chunk(k):
        t0 = k * CH
        for i in range(CH):
            t = t0 + i
            th, tr = divmod(t, 128 // 2)
            nc.tensor.matmul(
                out=ps_dots[:, t, :],
                lhsT=pts[:, t, :],
                rhs=cx.ct16a[:, NH * g + th, 2 * tr : 2 * tr + 2],
                start=(i == 0),
                stop=(i == CH - 1),
            )

    for k in range(NCH):
        t_chunk(k)
        if k >= LAG:
            d_chunk(k - LAG)
    for k in range(NCH - LAG, NCH):
        d_chunk(k)

    # valid dots sit at [row, parity=row//64]: extract the two halves
    dotw = cx.p_small.tile([128, NTILES], F32, tag="dotw", name=f"dotw{g}")
    nc.scalar.copy(out=dotw[0:64, :], in_=ps_dots[0:64, :, 0])
    nc.scalar.copy(out=dotw[64:128, :], in_=ps_dots[64:128, :, 1])

    # transpose [128(row), nt] -> [nt, 128(row)] => pair-major dots
    dr_ps = cx.ps_small.tile([NPAIR, 128], F32, tag="sm_ps", name=f"dr{g}")
    nc.tensor.transpose(out=dr_ps[:], in_=dotw[:], identity=cx.ident32[:])

    st.w = cx.p_small.tile([NPAIR, 2, N], F32, tag="w", name=f"w{g}")
    nc.vector.tensor_add(
        out=st.w[:],
        in0=dr_ps[:].rearrange("t (two n) -> t two n", two=2),
        in1=st.tw[:],
    )
    st.nmx = cx.p_small.tile([NPAIR, 2], F32, tag="nmx", name=f"nmx{g}")
    nc.vector.tensor_reduce(
        out=st.nmx[:], in_=st.w[:], axis=Axis.X, op=Alu.max, negate=True
    )
    _phase_b1(cx, st)
    return st


def _phase_b1(cx, st):
    """Softmax tail: exp + sum + reciprocal + normalize (no PE)."""
    nc = cx.nc
    g = st.g
    e = cx.p_small.tile([NPAIR, 2, N], F32, tag="e", name=f"e{g}")
    for s in range(2):
        nc.scalar.activation(
            out=e[:, s, :],
            in_=st.w[:, s, :],
            func=Act.Exp,
            bias=st.nmx[:, s : s + 1],
            scale=1.0,
        )
    z = cx.p_small.tile([NPAIR, 2], F32, tag="z", name=f"z{g}")
    nc.vector.reduce_sum(out=z[:], in_=e[:], axis=Axis.X)
    rz = cx.p_small.tile([NPAIR, 2], F32, tag="rz", name=f"rz{g}")
    nc.vector.reciprocal(out=rz[:], in_=z[:])
    st.wn16 = cx.p_small.tile([NPAIR, 2, N], F16, tag="wn16", name=f"wn{g}")
    for s in range(2):
        nc.scalar.activation(
            out=st.wn16[:, s, :],
            in_=e[:, s, :],
            func=Act.Copy,
            scale=rz[:, s : s + 1],
        )


def _phase_b2(cx, st):
    """W_MAT build: PE transpose of wn16 + block scatter."""
    nc = cx.nc
    g = st.g
    wc_ps = cx.ps_small.tile([128, NTILES], F16, tag="sm_ps", name=f"wc{g}")
    nc.tensor.transpose(
        out=wc_ps[:],
        in_=st.wn16[:].rearrange("t two n -> t (two n)"),
        identity=cx.ident16[0:NPAIR, 0:NPAIR],
    )
    wcol = cx.p_small.tile([128, NTILES], F16, tag="wcol", name=f"wcol{g}")
    nc.vector.tensor_copy(out=wcol[:], in_=wc_ps[:])
    # persistent pre-zeroed ring: only the data halves are ever written,
    # the zero halves survive across generations
    st.wmat16 = cx.wmat_ring[g % len(cx.wmat_ring)]
    nc.vector.tensor_copy(out=st.wmat16[0:64, :, 0], in_=wcol[0:64, :])
    nc.vector.tensor_copy(out=st.wmat16[64:128, :, 1], in_=wcol[64:128, :])


def _phase_c(cx, st):
    """Weighted-sum matmuls + u extraction (ACT) + store (sync)."""
    nc = cx.nc
    g = st.g
    b0 = g * GROUP
    HB = NTILES // 2
    for h in range(2):
        ps_ut = cx.ps_mm.tile(
            [128, HB, 2], F32, tag="mm_ps", name=f"ut{g}_{h}"
        )
        for k in range(HB):
            t = h * HB + k
            nc.tensor.matmul(
                out=ps_ut[:, k, :],
                lhsT=st.p16[:, t, :],
                rhs=st.wmat16[:, t, :],
                start=(k == 0),
                stop=(k == HB - 1),
            )
        uts = cx.p_small.tile(
            [128, GROUP // 2], F32, tag="uts", name=f"uts{g}_{h}"
        )
        nc.vector.tensor_copy(
            out=uts[:], in_=ps_ut[:].rearrange("d t two -> d (t two)")
        )
        bh = b0 + h * (GROUP // 2)
        nc.sync.dma_start(
            out=cx.u_all[:, bh : bh + GROUP // 2], in_=uts[:]
        )


def _emit_ct(cx, g):
    """PE transposes of group g's c halves into ct16a (+ scalar copies)."""
    nc = cx.nc
    for h in range(NH):
        gh = NH * g + h
        ct_ps = cx.ps_small.tile([128, 128], F16, tag="sm_ps", name=f"ct{gh}")
        nc.tensor.transpose(
            out=ct_ps[:],
            in_=cx.cg16[:, gh, :],
            identity=cx.ident16[:],
        )
        nc.vector.tensor_copy(out=cx.ct16a[:, gh, :], in_=ct_ps[:])


def _build_nc():
    nc = bass.Bass()
    pref = nc.declare_dram_parameter("pref", [BPC, N, D], F32, isOutput=False)
    c = nc.declare_dram_parameter("c", [BPC, 1, D], F32, isOutput=False)
    t_pref = nc.declare_dram_parameter("t_pref", [BPC, 1, N], F32, isOutput=False)
    t_c = nc.declare_dram_parameter("t_c", [BPC, 1], F32, isOutput=False)
    # u stored transposed [D, BPC] (direct from the weighted-sum PSUM
    # layout -- skips two PE transposes + copies per group); the host
    # transposes it back.
    u = nc.declare_dram_parameter("u", [D, BPC], F32, isOutput=True)

    pref_rows = pref[:].rearrange("b n d -> (b n) d")
    c_all = c[:].rearrange("b one d -> (b one) d")
    tp_all = t_pref[:].rearrange("b one n -> (b one) n")
    tc_all = t_c[:]
    u_all = u[:]

    with ExitStack() as ctx:
        tc = ctx.enter_context(tile.TileContext(nc))
        p_const = ctx.enter_context(tc.tile_pool(name="const", bufs=1))
        p_pre = ctx.enter_context(tc.tile_pool(name="pre", bufs=1))
        p_p16 = ctx.enter_context(tc.tile_pool(name="p16", bufs=4))
        p_pt = ctx.enter_context(tc.tile_pool(name="pt", bufs=3))
        p_small = ctx.enter_context(tc.tile_pool(name="small", bufs=3))
        ps_pt = ctx.enter_context(tc.tile_pool(name="ps_pt", bufs=4, space="PSUM"))
        ps_mm = ctx.enter_context(tc.tile_pool(name="ps_mm", bufs=3, space="PSUM"))
        ps_small = ctx.enter_context(
            tc.tile_pool(name="ps_small", bufs=1, space="PSUM")
        )

        nb = NGROUPS * GROUP

        # sync stream head: group 0's c first (contiguous 128 KB -- it
        # gates group 0's dot matmuls), then t tensors, then the rest of c.
        NGH = NGROUPS * NH
        c32a = p_pre.tile([128, NGH, D], F32)
        nc.sync.dma_start(
            out=c32a[:, 0:NH, :],
            in_=c_all[0:GROUP, :].rearrange("(h b) d -> b h d", b=128),
        )
        tpa = p_pre.tile([NPAIR, NGROUPS, 2, N], F32)
        nc.sync.dma_start(
            out=tpa[:],
            in_=tp_all[0:nb, :].rearrange(
                "(g t two) n -> t g two n", t=NPAIR, two=2
            ),
        )
        tca = p_pre.tile([NPAIR, NGROUPS, 2], F32)
        nc.sync.dma_start(
            out=tca[:],
            in_=tc_all[0:nb, :].rearrange(
                "(g t two) one -> t g (two one)", t=NPAIR, two=2
            ),
        )
        nc.sync.dma_start(
            out=c32a[:, NH:, :],
            in_=c_all[GROUP:nb, :].rearrange("(g b) d -> b g d", b=128),
        )

        ntca = p_pre.tile([NPAIR, NGROUPS, 2], F32)
        nc.vector.tensor_scalar_mul(out=ntca[:], in0=tca[:], scalar1=-1.0)

        # gpsimd stream: group 0's first pref chunk, then the identity
        # build (gpsimd-only affine_select) -- identities are ready right
        # when chunk 0's data lands.
        p16s = []
        for _gi in range(NGROUPS):
            p16_t = p_p16.tile(
                [128, NTILES, D], F16, tag="p16", name=f"p16_{_gi}"
            )
            p16s.append(p16_t)
        nc.gpsimd.dma_start(
            out=p16s[0][:, 0:16, :],
            in_=pref_rows[0 : 16 * 128, :].rearrange("(t p) d -> p t d", p=128),
        )
        ident16 = p_const.tile([128, 128], F16)
        ident32 = p_const.tile([128, 128], F32)
        make_identity(nc, ident16[:])
        make_identity(nc, ident32[:])
        consts = (ident16, ident32)

        # c cast: group 0 first, rest later (gates nothing early)
        cg16 = p_pre.tile([128, NGH, D], F16)
        nc.vector.tensor_copy(out=cg16[:, 0:NH, :], in_=c32a[:, 0:NH, :])
        nc.vector.tensor_copy(out=cg16[:, NH:, :], in_=c32a[:, NH:, :])
        ct16a = p_pre.tile([128, NGH, 128], F16)  # [D, group-half, batch]

        wmat_ring = []
        for _wi in range(3):
            wm = p_pre.tile([128, NTILES, 2], F16, name=f"wmatr{_wi}")
            nc.vector.memset(wm[:], 0.0)
            wmat_ring.append(wm)

        aps = (pref_rows, u_all, ct16a, tpa, tca, ntca, cg16, wmat_ring)
        cx = _Ctx(tc, (p_p16, p_pt, p_small, ps_pt, ps_mm, ps_small),
                  consts, aps)

        _emit_ct(cx, 0)

        # software pipeline, coarse blocks (fine interleaving costs ~10ns
        # of extra semaphore latency per matmul -- measured):
        #   iter g: [softmax-tail g-1] [A: loads+transposes+dots g]
        #           [W_MAT g-1] [weighted-sum + store g-1] [cT g+1]
        # Each deferred block's inputs are ready ~7 us before PE reaches
        # it, so no engine queue ever waits mid-chain.
        pend = None
        for g in range(NGROUPS):
            st = _phase_a(cx, g, p16s[g])
            if pend is not None:
                _phase_c(cx, pend)
            _phase_b2(cx, st)
            if g + 1 < NGROUPS:
                _emit_ct(cx, g + 1)
            pend = st

        _phase_c(cx, pend)

    return nc


_NC_CACHE = None
LAST_RESULT = None


def kernel(pref, c, t_pref, t_c):
    global _NC_CACHE, LAST_RESULT
    if _NC_CACHE is None:
        _NC_CACHE = _build_nc()
    nc = _NC_CACHE

    pref = np.ascontiguousarray(pref, dtype=np.float32)
    c = np.ascontiguousarray(c, dtype=np.float32)
    t_pref = np.ascontiguousarray(t_pref, dtype=np.float32)
    t_c = np.ascontiguousarray(t_c, dtype=np.float32)

    in_maps = []
    for i in range(NCORES):
        s = slice(i * BPC, (i + 1) * BPC)
        in_maps.append(
            {"pref": pref[s], "c": c[s], "t_pref": t_pref[s], "t_c": t_c[s]}
        )

    res = run_bass_kernel_spmd(nc, in_maps, list(range(NCORES)))
    LAST_RESULT = res
    return np.ascontiguousarray(
        np.concatenate([r["u"].T for r in res.results], axis=0)
    ).reshape(B, 1, D)



# revision 21
# speedup vs baseline: 1.0329x; 1.0328x over previous
"""Trainium2 Bass kernel for the AggregateLayer pooling problem.

reference semantics (per batch b):
    dot_w[j] = <pref[b,j,:], c[b,0,:]>                      (j = 0..63)
    t_w[j]   = 1 / |t_pref[b,0,j] - t_c[b,0]|
    w        = softmax(dot_w + t_w)                          (over j)
    u[b,0,:] = sum_j w[j] * pref[b,j,:]

Strategy: pure data parallel over 8 NeuronCores (1024 batches each).
Per core, batches are processed in groups of GROUP=128 (NTILES=64 tiles
of 2 batches; a tile is the 128 flattened (batch, j) rows x 128 D cols).

The kernel is HBM-bandwidth-bound: the pref stream (33.5 MB fp32/core)
runs at the ~360 GB/s per-core HBM cap (~95 us of pure streaming).
Everything else is engineered to hide under that stream:
  - pref chunk loads (cast fp32->fp16 in SWDGE) are the only traffic on
    the gpsimd ring; c/t_pref/t_c loads and u stores ride sync HWDGE.
  - group 0's c slice loads first (contiguous 64 KB) so its dot matmuls
    aren't gated; later groups' c transposes are emitted lazily; the
    identity build (gpsimd-only) slots behind group 0's first chunk.
  - engine streams are in-order, so emission is software-pipelined at
    COARSE block granularity (fine interleaving measurably costs ~10ns
    of semaphore latency per matmul): dot-matmul chunks trail the
    transpose chunks by LAG=3 (both paced by the PSUM->SBUF pts copies,
    split 5:3 ACT/DVE), and each group's weighted sum is deferred one
    full group so PE chews on group g+1's transposes while group g's
    softmax runs on DVE/ACT.
  - u is stored transposed [D, BPC] straight from the weighted-sum PSUM
    layout (saves two PE transposes + copies per group); the host
    transposes it back. W_MAT lives in a persistent pre-zeroed 3-ring.
  - t_w is computed on ACT (Abs with bias=-t_c, ntca precomputed) and
    wn16 on ACT (Copy with per-partition scale=rz), freeing ~2us/group
    of DVE time on the softmax->W_MAT critical chain.

Approaches measured SLOWER this hardware/stack (do not retry blindly):
  - XBAR dma_start_transpose (sync HWDGE) to replace PE transposes:
    the runtime serializes HWDGE XPOSEs against the SWDGE pref stream
    with ~2-12us ring handoffs (and concurrent XPOSEs on the two HWDGE
    queues corrupt each other's output). Any XPOSE/SWDGE mix ran
    224-324us. XPOSE alone streams at ~190 GB/s per queue.
  - batch-on-partitions dataflow (dots/softmax/wsum as within-partition
    DVE ops with stride-0 broadcast APs): DVE runs 1x (not 2x) on
    inner-stride-0 broadcast operands and on tensor_reduce, so a
    128-batch block costs ~26us of DVE vs ~11us of PE on the row path;
    a 6/2 row/batch split measured 155us.
  - tiny N=2 matmuls cost ~36ns back-to-back (LDWEIGHTS pipelines);
    PE transposes ~95ns; the SWDGE pref stream delivers ~712 GB/s.

Tuning cliffs measured on hardware (do not "fix" these without
re-measuring): the last group's HT=8 DMA chunking is load-bearing
(HT=16 there reproducibly ~2x-es runtime via p16 buffer-ring
interaction); per-half softmax chains, fully-upfront DMA emission, and
fp32-bitcast PSUM copies (PSUM fp16 is not packed-viewable) all
regressed. Run-to-run noise is +-3 us with occasional +15-40 us
device-state excursions -- attribute changes only across repeated runs.
"""

import numpy as np
from contextlib import ExitStack

import concourse.bass as bass
import concourse.tile as tile
from concourse import mybir
from concourse.masks import make_identity
from concourse.bass_utils import run_bass_kernel_spmd
import concourse.bass2jax as _b2j


def _split_multiwait(bir: dict) -> int:
    """Walrus in this container rejects >1 sync-wait per instruction.

    Hoist excess waits onto NoOps inserted just before the instruction on
    the same engine (program order within the engine stream preserves the
    wait semantics exactly).
    """
    n = 0
    for fn in bir["functions"]:
        for blk in fn["blocks"]:
            out = []
            for inst in blk["instructions"]:
                si = inst.get("sync_info")
                waits = si.get("on_wait") if si else None
                if waits and len(waits) > 1:
                    for w in waits[:-1]:
                        out.append(
                            {
                                "opcode": "NoOp",
                                "engine": inst["engine"],
                                "name": f"{inst['name']}-xw{n}",
                                "ins": [],
                                "outs": [],
                                "sync_info": {"on_update": [], "on_wait": [w]},
                            }
                        )
                        n += 1
                    si["on_wait"] = [waits[-1]]
                out.append(inst)
            blk["instructions"] = out
    return n


_orig_compile_bir_kernel = _b2j.compile_bir_kernel


def _legalizing_compile_bir_kernel(ant_bir_str, *args, **kwargs):
    import orjson

    bir = orjson.loads(ant_bir_str)
    _split_multiwait(bir)
    return _orig_compile_bir_kernel(orjson.dumps(bir), *args, **kwargs)


_b2j.compile_bir_kernel = _legalizing_compile_bir_kernel

F32 = mybir.dt.float32
F16 = mybir.dt.float16
Alu = mybir.AluOpType
Act = mybir.ActivationFunctionType
Axis = mybir.AxisListType

B, N, D = 8192, 64, 128
NCORES = 8
BPC = B // NCORES          # 1024 batches per core
GROUP = 128                # batches per group
NGROUPS = BPC // GROUP     # 8
NTILES = GROUP // 2        # 64 two-batch tiles per group
NPAIR = GROUP // 2         # 64 batch-pairs per group (softmax partitions)
NH = GROUP // 128          # 1 c-half per group
CH = 8                     # tiles per transpose/copy chunk
NCH = NTILES // CH         # 8 chunks per group
LAG = 3                    # dot-chunk k trails transpose-chunk k+LAG


class _St:
    """Per-group pipeline state carried between build phases."""

    def __init__(self, g):
        self.g = g
        self.p16 = None
        self.tw = None
        self.w = None
        self.nmx = None
        self.wn16 = None
        self.wmat16 = None


class _Ctx:
    def __init__(self, tc, pools, consts, aps):
        self.tc = tc
        self.nc = tc.nc
        (self.p_p16, self.p_pt, self.p_small, self.ps_pt, self.ps_mm,
         self.ps_small) = pools
        self.ident16, self.ident32 = consts
        (self.pref_rows, self.u_all, self.ct16a, self.tpa, self.tca,
         self.ntca, self.cg16, self.wmat_ring) = aps


def _phase_a(cx, g, p16):
    """tw + interleaved transpose/dot passes + dots extraction + add/max."""
    nc = cx.nc
    st = _St(g)
    st.p16 = p16

    # pref chunk DMAs for this group (group 0's first chunk was emitted
    # ahead of the identity build in _build_nc); last group uses smaller
    # chunks so the drain after the final chunk is short
    r0 = g * GROUP * N
    HT = 16 if g < NGROUPS - 1 else 8
    for h0 in range(16 if g == 0 else 0, NTILES, HT):
        rh = r0 + h0 * 128
        nc.gpsimd.dma_start(
            out=p16[:, h0 : h0 + HT, :],
            in_=cx.pref_rows[rh : rh + HT * 128, :].rearrange(
                "(t p) d -> p t d", p=128
            ),
        )

    st.tw = cx.p_small.tile([NPAIR, 2, N], F32, tag="tw", name=f"tw{g}")
    for s in range(2):
        nc.scalar.activation(
            out=st.tw[:, s, :],
            in_=cx.tpa[:, g, s, :],
            func=Act.Abs,
            bias=cx.ntca[:, g, s : s + 1],
            scale=1.0,
        )
    nc.vector.reciprocal(out=st.tw[:], in_=st.tw[:])

    pts = cx.p_pt.tile([128, NTILES, 128], F16, tag="pts", name=f"pts{g}")
    ps_dots = cx.ps_mm.tile(
        [128, NTILES, 2], F32, tag="mm_ps", name=f"dots{g}"
    )

    def t_chunk(k):
        t0 = k * CH
        pt_ps = cx.ps_pt.tile(
            [128, CH, 128], F16, tag="pt_ps", name=f"ptps{g}_{k}"
        )
        for i in range(CH):
            nc.tensor.transpose(
                out=pt_ps[:, i, :],
                in_=p16[:, t0 + i, :],
                identity=cx.ident16[:],
            )
        if k < 5:
            nc.scalar.copy(out=pts[:, t0 : t0 + CH, :], in_=pt_ps[:])
        else:
            nc.vector.tensor_copy(out=pts[:, t0 : t0 + CH, :], in_=pt_ps[:])

    def d_chunk(k):
        t0 = k * CH
        for i in range(CH):
            t = t0 + i
            th, tr = divmod(t, 128 // 2)
            nc.tensor.matmul(
                out=ps_dots[:, t, :],
                lhsT=pts[:, t, :],
                rhs=cx.ct16a[:, NH * g + th, 2 * tr : 2 * tr + 2],
                start=(i == 0),
                stop=(i == CH - 1),
            )

    for k in range(NCH):
        t_chunk(k)
        if k >= LAG:
            d_chunk(k - LAG)
    for k in range(NCH - LAG, NCH):
        d_chunk(k)

    # valid dots sit at [row, parity=row//64]: extract the two halves
    dotw = cx.p_small.tile([128, NTILES], F32, tag="dotw", name=f"dotw{g}")
    nc.scalar.copy(out=dotw[0:64, :], in_=ps_dots[0:64, :, 0])
    nc.scalar.copy(out=dotw[64:128, :], in_=ps_dots[64:128, :, 1])

    # transpose [128(row), nt] -> [nt, 128(row)] => pair-major dots
    dr_ps = cx.ps_small.tile([NPAIR, 128], F32, tag="sm_ps", name=f"dr{g}")
    nc.tensor.transpose(out=dr_ps[:], in_=dotw[:], identity=cx.ident32[:])

    st.w = cx.p_small.tile([NPAIR, 2, N], F32, tag="w", name=f"w{g}")
    nc.vector.tensor_add(
        out=st.w[:],
        in0=dr_ps[:].rearrange("t (two n) -> t two n", two=2),
        in1=st.tw[:],
    )
    st.nmx = cx.p_small.tile([NPAIR, 2], F32, tag="nmx", name=f"nmx{g}")
    nc.vector.tensor_reduce(
        out=st.nmx[:], in_=st.w[:], axis=Axis.X, op=Alu.max, negate=True
    )
    _phase_b1(cx, st)
    return st


def _phase_b1(cx, st):
    """Softmax tail: exp + sum + reciprocal + normalize (no PE)."""
    nc = cx.nc
    g = st.g
    e = cx.p_small.tile([NPAIR, 2, N], F32, tag="e", name=f"e{g}")
    for s in range(2):
        nc.scalar.activation(
            out=e[:, s, :],
            in_=st.w[:, s, :],
            func=Act.Exp,
            bias=st.nmx[:, s : s + 1],
            scale=1.0,
        )
    z = cx.p_small.tile([NPAIR, 2], F32, tag="z", name=f"z{g}")
    nc.vector.reduce_sum(out=z[:], in_=e[:], axis=Axis.X)
    rz = cx.p_small.tile([NPAIR, 2], F32, tag="rz", name=f"rz{g}")
    nc.vector.reciprocal(out=rz[:], in_=z[:])
    st.wn16 = cx.p_small.tile([NPAIR, 2, N], F16, tag="wn16", name=f"wn{g}")
    for s in range(2):
        nc.scalar.activation(
            out=st.wn16[:, s, :],
            in_=e[:, s, :],
            func=Act.Copy,
            scale=rz[:, s : s + 1],
        )


def _phase_b2(cx, st):
    """W_MAT build: PE transpose of wn16 + block scatter."""
    nc = cx.nc
    g = st.g
    wc_ps = cx.ps_small.tile([128, NTILES], F16, tag="sm_ps", name=f"wc{g}")
    nc.tensor.transpose(
        out=wc_ps[:],
        in_=st.wn16[:].rearrange("t two n -> t (two n)"),
        identity=cx.ident16[0:NPAIR, 0:NPAIR],
    )
    wcol = cx.p_small.tile([128, NTILES], F16, tag="wcol", name=f"wcol{g}")
    nc.vector.tensor_copy(out=wcol[:], in_=wc_ps[:])
    # persistent pre-zeroed ring: only the data halves are ever written,
    # the zero halves survive across generations
    st.wmat16 = cx.wmat_ring[g % len(cx.wmat_ring)]
    nc.vector.tensor_copy(out=st.wmat16[0:64, :, 0], in_=wcol[0:64, :])
    nc.vector.tensor_copy(out=st.wmat16[64:128, :, 1], in_=wcol[64:128, :])


def _phase_c(cx, st):
    """Weighted-sum matmuls + u extraction (ACT) + store (sync)."""
    nc = cx.nc
    g = st.g
    b0 = g * GROUP
    HB = NTILES // 2
    for h in range(2):
        ps_ut = cx.ps_mm.tile(
            [128, HB, 2], F32, tag="mm_ps", name=f"ut{g}_{h}"
        )
        for k in range(HB):
            t = h * HB + k
            nc.tensor.matmul(
                out=ps_ut[:, k, :],
                lhsT=st.p16[:, t, :],
                rhs=st.wmat16[:, t, :],
                start=(k == 0),
                stop=(k == HB - 1),
            )
        uts = cx.p_small.tile(
            [128, GROUP // 2], F32, tag="uts", name=f"uts{g}_{h}"
        )
        nc.vector.tensor_copy(
            out=uts[:], in_=ps_ut[:].rearrange("d t two -> d (t two)")
        )
        bh = b0 + h * (GROUP // 2)
        nc.sync.dma_start(
            out=cx.u_all[:, bh : bh + GROUP // 2], in_=uts[:]
        )


def _emit_ct(cx, g):
    """PE transposes of group g's c halves into ct16a (+ scalar copies)."""
    nc = cx.nc
    for h in range(NH):
        gh = NH * g + h
        ct_ps = cx.ps_small.tile([128, 128], F16, tag="sm_ps", name=f"ct{gh}")
        nc.tensor.transpose(
            out=ct_ps[:],
            in_=cx.cg16[:, gh, :],
            identity=cx.ident16[:],
        )
        nc.vector.tensor_copy(out=cx.ct16a[:, gh, :], in_=ct_ps[:])


def _build_nc():
    nc = bass.Bass()
    pref = nc.declare_dram_parameter("pref", [BPC, N, D], F32, isOutput=False)
    c = nc.declare_dram_parameter("c", [BPC, 1, D], F32, isOutput=False)
    t_pref = nc.declare_dram_parameter("t_pref", [BPC, 1, N], F32, isOutput=False)
    t_c = nc.declare_dram_parameter("t_c", [BPC, 1], F32, isOutput=False)
    # u stored transposed [D, BPC] (direct from the weighted-sum PSUM
    # layout -- skips two PE transposes + copies per group); the host
    # transposes it back.
    u = nc.declare_dram_parameter("u", [D, BPC], F32, isOutput=True)

    pref_rows = pref[:].rearrange("b n d -> (b n) d")
    c_all = c[:].rearrange("b one d -> (b one) d")
    tp_all = t_pref[:].rearrange("b one n -> (b one) n")
    tc_all = t_c[:]
    u_all = u[:]

    with ExitStack() as ctx:
        tc = ctx.enter_context(tile.TileContext(nc))
        p_const = ctx.enter_context(tc.tile_pool(name="const", bufs=1))
        p_pre = ctx.enter_context(tc.tile_pool(name="pre", bufs=1))
        p_p16 = ctx.enter_context(tc.tile_pool(name="p16", bufs=4))
        p_pt = ctx.enter_context(tc.tile_pool(name="pt", bufs=3))
        p_small = ctx.enter_context(tc.tile_pool(name="small", bufs=3))
        ps_pt = ctx.enter_context(tc.tile_pool(name="ps_pt", bufs=3, space="PSUM"))
        ps_mm = ctx.enter_context(tc.tile_pool(name="ps_mm", bufs=3, space="PSUM"))
        ps_small = ctx.enter_context(
            tc.tile_pool(name="ps_small", bufs=2, space="PSUM")
        )

        nb = NGROUPS * GROUP

        # sync stream head: group 0's c first (contiguous 128 KB -- it
        # gates group 0's dot matmuls), then t tensors, then the rest of c.
        NGH = NGROUPS * NH
        c32a = p_pre.tile([128, NGH, D], F32)
        nc.sync.dma_start(
            out=c32a[:, 0:NH, :],
            in_=c_all[0:GROUP, :].rearrange("(h b) d -> b h d", b=128),
        )
        tpa = p_pre.tile([NPAIR, NGROUPS, 2, N], F32)
        nc.sync.dma_start(
            out=tpa[:],
            in_=tp_all[0:nb, :].rearrange(
                "(g t two) n -> t g two n", t=NPAIR, two=2
            ),
        )
        tca = p_pre.tile([NPAIR, NGROUPS, 2], F32)
        nc.sync.dma_start(
            out=tca[:],
            in_=tc_all[0:nb, :].rearrange(
                "(g t two) one -> t g (two one)", t=NPAIR, two=2
            ),
        )
        nc.sync.dma_start(
            out=c32a[:, NH:, :],
            in_=c_all[GROUP:nb, :].rearrange("(g b) d -> b g d", b=128),
        )

        ntca = p_pre.tile([NPAIR, NGROUPS, 2], F32)
        nc.vector.tensor_scalar_mul(out=ntca[:], in0=tca[:], scalar1=-1.0)

        # gpsimd stream: group 0's first pref chunk, then the identity
        # build (gpsimd-only affine_select) -- identities are ready right
        # when chunk 0's data lands.
        p16s = []
        for _gi in range(NGROUPS):
            p16_t = p_p16.tile(
                [128, NTILES, D], F16, tag="p16", name=f"p16_{_gi}"
            )
            p16s.append(p16_t)
        nc.gpsimd.dma_start(
            out=p16s[0][:, 0:16, :],
            in_=pref_rows[0 : 16 * 128, :].rearrange("(t p) d -> p t d", p=128),
        )
        ident16 = p_const.tile([128, 128], F16)
        ident32 = p_const.tile([128, 128], F32)
        make_identity(nc, ident16[:])
        make_identity(nc, ident32[:])
        consts = (ident16, ident32)

        # c cast: group 0 first, rest later (gates nothing early)
        cg16 = p_pre.tile([128, NGH, D], F16)
        nc.vector.tensor_copy(out=cg16[:, 0:NH, :], in_=c32a[:, 0:NH, :])
        nc.vector.tensor_copy(out=cg16[:, NH:, :], in_=c32a[:, NH:, :])
        ct16a = p_pre.tile([128, NGH, 128], F16)  # [D, group-half, batch]

        wmat_ring = []
        for _wi in range(3):
            wm = p_pre.tile([128, NTILES, 2], F16, name=f"wmatr{_wi}")
            nc.vector.memset(wm[:], 0.0)
            wmat_ring.append(wm)

        aps = (pref_rows, u_all, ct16a, tpa, tca, ntca, cg16, wmat_ring)
        cx = _Ctx(tc, (p_p16, p_pt, p_small, ps_pt, ps_mm, ps_small),
                  consts, aps)

        _emit_ct(cx, 0)

        # software pipeline, coarse blocks (fine interleaving costs ~10ns
        # of extra semaphore latency per matmul -- measured):
        #   iter g: [softmax-tail g-1] [A: loads+transposes+dots g]
        #           [W_MAT g-1] [weighted-sum + store g-1] [cT g+1]
        # Each deferred block's inputs are ready ~7 us before PE reaches
        # it, so no engine queue ever waits mid-chain.
        pend = None
        for g in range(NGROUPS):
            st = _phase_a(cx, g, p16s[g])
            if pend is not None:
                _phase_c(cx, pend)
            _phase_b2(cx, st)
            if g + 1 < NGROUPS:
                _emit_ct(cx, g + 1)
            pend = st

        _phase_c(cx, pend)

    return nc


_NC_CACHE = None
LAST_RESULT = None


def kernel(pref, c, t_pref, t_c):
    global _NC_CACHE, LAST_RESULT
    if _NC_CACHE is None:
        _NC_CACHE = _build_nc()
    nc = _NC_CACHE

    pref = np.ascontiguousarray(pref, dtype=np.float32)
    c = np.ascontiguousarray(c, dtype=np.float32)
    t_pref = np.ascontiguousarray(t_pref, dtype=np.float32)
    t_c = np.ascontiguousarray(t_c, dtype=np.float32)

    in_maps = []
    for i in range(NCORES):
        s = slice(i * BPC, (i + 1) * BPC)
        in_maps.append(
            {"pref": pref[s], "c": c[s], "t_pref": t_pref[s], "t_c": t_c[s]}
        )

    res = run_bass_kernel_spmd(nc, in_maps, list(range(NCORES)))
    LAST_RESULT = res
    return np.ascontiguousarray(
        np.concatenate([r["u"].T for r in res.results], axis=0)
    ).reshape(B, 1, D)

